# revision 3
# baseline (speedup 1.0000x reference)
"""Involution2d (B=8, C=256, H=W=56, K=7, G=16, reduction=4) on 8 TRN2 cores.

Spatial shard over H (7 output rows + 3-row halos per core), involution
partition layout = (b, g) = 128. This environment's per-rep cost is dominated
by a large fixed per-instruction latency (~0.2 ms, engines effectively
serialized), so the kernel minimizes raw instruction count (~100/rep vs ~313
in v1):
  - both biases folded into stage-2: b2eff = w_span@b_reduce + b_span enters
    the matmul as a 65th contraction row against a ones-row in z.
  - one 7-bank psum tile [112, 3584] shared by stage-1 (rows 0:64) and the
    7 stage-2 chunks; one activation copy per chunk (8 copies total).
  - ker transpose to involution layout: per chunk ONE spill to DRAM in
    (b, g, kw, p) order + ONE fully-contiguous gather back (partition-
    crossing SBUF->SBUF DMAs are not expressible, so DRAM bounce).
  - involution per kh: ONE fused mul into planes 1-7 of a [128, 9, 7168]
    tile (overlapping-stride x AP, per-(b,g) tap broadcast over c') + ONE
    8-plane strided tensor_reduce that also folds in the previous bf16
    accumulator (acc ping-pongs between planes 0 and 8) -> 2 vector ops/kh.
  - stage-1 rhs (xsmm) is host-precast to bf16 in DoubleRow-friendly
    [p, i, n] layout and loaded once; output stores bf16 acc with a casting
    gpsimd DMA.
"""

import os
import sys

import numpy as np

for _p in ("/opt/trn_rl_repo",):
    if os.path.isdir(_p) and _p not in sys.path:
        sys.path.insert(0, _p)

import bass_rust
import concourse.bacc as bacc
import concourse.mybir as mybir
from concourse.tile import TileContext
from concourse.bass_utils import run_bass_kernel_spmd

B, C, H, W = 8, 256, 56, 56
G, K, PAD = 16, 7, 3
CPG = C // G            # 16
KK = K * K
CR = 64
NCORES = 8
HS = H // NCORES        # 7 rows per core
HALO = PAD
HP = HS + 2 * HALO      # 13
LPAD = 4
WP = 64                 # 4 + 56 + 4
NPIX = HS * WP          # 448
NALLP = B * NPIX        # 3584
CROW = HP * WP          # 832
XFLAT = CPG * CROW      # 13312
XOFF = 8
XPAD = 13712            # XOFF + XFLAT + max shift (387) + slack
NF = CPG * NPIX         # 7168

F32 = mybir.dt.float32
BF16 = mybir.dt.bfloat16

MCHUNK = G * K          # 112
NCHUNKS = K
PRODF = K * NF          # 50176

USE_DR = False  # DoubleRow is fp8-only on TRN2


def _build(reps=1):
    nc = bacc.Bacc(trn_type="TRN2")

    xs = nc.dram_tensor("xs", [B, C, HP, WP], F32, kind="ExternalInput").ap()
    # DoubleRow layout: xsmm_dr[p, i, n] = x_flat[i*128+p, n], bf16 host-precast
    xsmm = nc.dram_tensor("xsmm", [128, 2 * NALLP], BF16, kind="ExternalInput").ap()
    # w1dr[p, i, m] = w_reduce[m, i*128+p]
    w1d = nc.dram_tensor("w1d", [128, 2 * CR], F32, kind="ExternalInput").ap()
    # rows 0:64 = w_span[perm].T ; row 64 = b2eff[perm]
    w2t = nc.dram_tensor("w2t", [CR + 1, G * KK], F32, kind="ExternalInput").ap()
    out = nc.dram_tensor("out", [128, NF], F32, kind="ExternalOutput").ap()
    # ker bounce scratch in involution order (b, g, kw, p)
    kscr = nc.dram_tensor("kscr", [reps, NCHUNKS, B, G, K, NPIX], BF16).ap()

    with TileContext(nc) as tc:
        with (
            tc.tile_pool(name="const", bufs=1) as cpool,
            tc.tile_pool(name="xp", bufs=1) as xpool,
            tc.tile_pool(name="work", bufs=1) as wpool,
            tc.tile_pool(name="stage", bufs=2) as spool,
            tc.tile_pool(name="psum", bufs=1, space="PSUM") as ppool,
        ):
            # ---------------- weights ----------------
            w1 = cpool.tile([128, 2 * CR], BF16, tag="w1", name="w1")
            nc.gpsimd.dma_start(out=w1[:, :], in_=w1d)
            w1v = w1[:, :].rearrange("p (i m) -> p i m", i=2)

            w2all = cpool.tile([CR + 1, G * KK], BF16, tag="w2", name="w2all")
            nc.gpsimd.dma_start(out=w2all[:, :], in_=w2t)

            # ---------------- x slab (involution layout, partitions=(b,g)) ----
            x_even = xpool.tile([128, XPAD], BF16, tag="xe", name="x_even")
            xs_g = xs.rearrange("b (g c) h w -> (b g) (c h w)", g=G)
            nc.vector.memset(x_even[:, :], 0.0)
            nc.gpsimd.dma_start(out=x_even[:, XOFF:XOFF + XFLAT], in_=xs_g)

            # ---------------- persistent tiles ----------------
            z_sb = wpool.tile([CR + 1, NALLP], BF16, tag="z", name="z_sb")
            nc.vector.memset(z_sb[CR:CR + 1, :], 1.0)
            # 9 planes: 0/8 = bf16 acc ping-pong, 1-7 = per-kw products
            prod = wpool.tile([128, 9 * NF], BF16, tag="prod", name="prod")
            xmm_t = wpool.tile([128, 2 * NALLP], BF16, tag="xmm", name="xmm")
            nc.sync.dma_start(out=xmm_t[:, :], in_=xsmm)
            xmm_v = xmm_t[:, :].rearrange("p (i n) -> p i n", i=2)

            # one 7-bank psum tile; one copy per chunk minimizes instruction
            # count (engines appear serialized in this environment)
            psF = ppool.tile([MCHUNK, NALLP], F32, tag="psf", name="psF")
            SPLITS7 = tuple((a, min(a + 512, NALLP)) for a in range(0, NALLP, 512))

            # custom overlapping AP helper for the fused involution mul
            x0 = x_even[:, 0:1]
            part_pair = x_even[:, :].ap[0]  # [stride, 128]

            def xin_ap(kh):
                base = XOFF - PAD + kh * WP
                return bass_rust.AP(
                    tensor=x0.tensor,
                    offset=x0.offset + base,
                    ap=[list(part_pair), [1, K], [CROW, CPG], [1, NPIX]],
                )

            for rep in range(reps):
                # ---------------- z = w1.T @ x ----------------
                for (a, b_) in SPLITS7:
                    for i in range(2):
                        nc.tensor.matmul(
                            out=psF[0:CR, a:b_],
                            lhsT=w1v[:, i, :],
                            rhs=xmm_v[:, i, a:b_],
                            start=(i == 0),
                            stop=(i == 1),
                        )
                nc.scalar.copy(z_sb[0:CR, :], psF[0:CR, :])

                # ---------------- ker chunks + gather ----------------
                ktaps = []
                for j in range(NCHUNKS):
                    lhsT2 = w2all[:, j * MCHUNK:(j + 1) * MCHUNK]
                    kst = spool.tile(
                        [MCHUNK, NALLP], BF16, tag="kst", bufs=2,
                        name=f"kst{rep}_{j}",
                    )
                    for (a, b_) in SPLITS7:
                        nc.tensor.matmul(
                            out=psF[:, a:b_],
                            lhsT=lhsT2,
                            rhs=z_sb[:, a:b_],
                            start=True,
                            stop=True,
                        )
                    nc.scalar.copy(kst[:, :], psF[:, :])

                    # one spill to DRAM in involution order (b,g,kw,p),
                    # one fully-contiguous gather back to partitions (b,g)
                    eng_s = nc.sync if j % 2 == 0 else nc.scalar
                    eng_g = nc.scalar if j % 2 == 0 else nc.sync
                    eng_s.dma_start(
                        out=kscr[rep, j].rearrange("b g kw p -> (g kw) b p"),
                        in_=kst[:, :].rearrange("a (b p) -> a b p", b=B),
                    )
                    ktap = spool.tile(
                        [128, K * NPIX], BF16, tag="ktap", bufs=2,
                        name=f"ktap{rep}_{j}",
                    )
                    eng_g.dma_start(
                        out=ktap[:, :],
                        in_=kscr[rep, j].rearrange("b g kw p -> (b g) (kw p)"),
                    )
                    ktaps.append(ktap)

                # ---------------- involution ----------------
                # products go to planes 1-7; bf16 acc ping-pongs 0 <-> 8 via
                # an 8-plane strided reduce (products + previous acc)
                with nc.allow_low_precision("involution bf16 partials"):
                    for kh in range(K):
                        ktap = ktaps[kh]
                        kv = (
                            ktap[:, :]
                            .rearrange("p (kw r) -> p kw r", kw=K)
                            .unsqueeze(2)
                            .broadcast_to((128, K, CPG, NPIX))
                        )
                        pv = prod[:, NF:8 * NF].rearrange(
                            "p (kw c r) -> p kw c r", kw=K, c=CPG
                        )
                        nc.vector.tensor_mul(pv, xin_ap(kh), kv)
                        if kh == 0:
                            rin = prod[:, NF:8 * NF].rearrange(
                                "p (kw f) -> p f kw", kw=K
                            )
                            ro = prod[:, 8 * NF:9 * NF]
                        elif kh % 2 == 1:
                            rin = prod[:, NF:9 * NF].rearrange(
                                "p (kw f) -> p f kw", kw=8
                            )
                            ro = prod[:, 0:NF]
                        else:
                            rin = prod[:, 0:8 * NF].rearrange(
                                "p (kw f) -> p f kw", kw=8
                            )
                            ro = prod[:, 8 * NF:9 * NF]
                        nc.vector.tensor_reduce(
                            out=ro, in_=rin,
                            axis=mybir.AxisListType.X,
                            op=mybir.AluOpType.add,
                        )

                # ---------------- store (bf16 acc plane 8 -> f32, cast) ----
                nc.gpsimd.dma_start(out=out, in_=prod[:, 8 * NF:9 * NF])

    return nc


_CACHE = {}


def _get_program(reps=1):
    if reps not in _CACHE:
        nc = _build(reps)
        nc.compile()
        _CACHE[reps] = nc
    return _CACHE[reps]


def _make_inputs(x, w_reduce, b_reduce, w_span, b_span):
    import ml_dtypes

    x = np.ascontiguousarray(np.asarray(x, dtype=np.float32))
    w_reduce = np.asarray(w_reduce, np.float32)
    b_reduce = np.asarray(b_reduce, np.float32)
    w_span = np.asarray(w_span, np.float32)
    b_span = np.asarray(b_span, np.float32)

    # w1 DoubleRow layout: w1d[p, i*CR + m] = w_reduce[m, i*128+p]
    w1d = np.ascontiguousarray(
        w_reduce.T.reshape(2, 128, CR).transpose(1, 0, 2).reshape(128, 2 * CR)
    )

    # tap-major permutation: col j*112 + g*7 + kk <- w_span row g*49 + j*7 + kk
    perm = np.empty(G * KK, np.int64)
    idx = 0
    for j in range(NCHUNKS):
        for g in range(G):
            for kk in range(K):
                perm[idx] = g * KK + j * K + kk
                idx += 1
    b2eff = w_span @ b_reduce + b_span
    w2te = np.concatenate([w_span[perm].T, b2eff[perm][None, :]], axis=0)
    w2te = np.ascontiguousarray(w2te)

    in_maps = []
    for i in range(NCORES):
        h0 = i * HS - HALO
        sl = np.zeros((B, C, HP, WP), np.float32)
        s0, s1 = max(0, h0), min(H, h0 + HP)
        sl[:, :, s0 - h0:s1 - h0, LPAD:LPAD + W] = x[:, :, s0:s1, :]
        xf = sl[:, :, HALO:HALO + HS, :].transpose(1, 0, 2, 3).reshape(C, NALLP)
        # DoubleRow rhs layout: [p, i*NALLP + n] = xf[i*128+p, n], bf16
        xdr = np.ascontiguousarray(
            xf.reshape(2, 128, NALLP).transpose(1, 0, 2).reshape(128, 2 * NALLP)
        ).astype(ml_dtypes.bfloat16)
        in_maps.append({"xs": sl, "xsmm": xdr, "w1d": w1d, "w2t": w2te})
    return in_maps


def _unpack_out(arr):
    """[128=(b,g), NF=(c',hs,wp)] fp32 -> [B, C, HS, W]"""
    a = arr.reshape(B, C, HS, WP)[:, :, :, LPAD:LPAD + W]
    return np.ascontiguousarray(a)


def kernel_with_results(x, w_reduce, b_reduce, w_span, b_span, trace=False, reps=1):
    in_maps = _make_inputs(x, w_reduce, b_reduce, w_span, b_span)
    nc = _get_program(reps)
    res = run_bass_kernel_spmd(nc, in_maps, list(range(NCORES)), trace=trace)
    full = np.concatenate(
        [_unpack_out(res.results[i]["out"]) for i in range(NCORES)], axis=2
    ).astype(np.float32)
    return full, res


def kernel(x, w_reduce, b_reduce, w_span, b_span):
    full, _ = kernel_with_results(x, w_reduce, b_reduce, w_span, b_span)
    return full


# revision 4
# speedup vs baseline: 1.5210x; 1.5210x over previous
"""Involution2d (B=8, C=256, H=W=56, K=7, G=16, reduction=4) on 8 TRN2 cores, v2.

Same layout as v1 (spatial shard over H, partitions=(g,b), padded width 64) but
restructured to minimize instruction count and dependency depth:
  - stage-1 bias folded into stage-2 via b2eff = w_span@b_reduce + b_span;
    stage-2 bias folded into the matmul via a ones-row in z (contraction 65).
  - stage-1 uses DoubleRow perf mode: contraction 256 in one matmul -> 7 mm.
  - stage-2: 7 chunks x 7 matmuls into 2048+1536 psum tiles, 2 copies each.
  - ktap gather is a single SBUF->SBUF DMA per (chunk, half) - no DRAM trip.
  - involution per kh: ONE fused mul [128,7,16,448] with an overlapping
    x AP + ONE strided tensor_reduce over kw + acc add  (3 vector ops/kh
    instead of 15).
  - xmm (stage-1 rhs, bf16 host-precast) aliases the prod buffer and is
    reloaded per rep (1 contiguous DMA).
"""

import os
import sys

import numpy as np

for _p in ("/opt/trn_rl_repo",):
    if os.path.isdir(_p) and _p not in sys.path:
        sys.path.insert(0, _p)

import bass_rust
import concourse.bacc as bacc
import concourse.mybir as mybir
from concourse.tile import TileContext
from concourse.bass_utils import run_bass_kernel_spmd

B, C, H, W = 8, 256, 56, 56
G, K, PAD = 16, 7, 3
CPG = C // G            # 16
KK = K * K
CR = 64
NCORES = 8
HS = H // NCORES        # 7 rows per core
HALO = PAD
HP = HS + 2 * HALO      # 13
LPAD = 4
WP = 64                 # 4 + 56 + 4
NPIX = HS * WP          # 448
NALLP = B * NPIX        # 3584
CROW = HP * WP          # 832
XFLAT = CPG * CROW      # 13312
XOFF = 8
XPAD = 13712            # XOFF + XFLAT + max shift (387) + slack
NF = CPG * NPIX         # 7168

F32 = mybir.dt.float32
BF16 = mybir.dt.bfloat16

MCHUNK = G * K          # 112
NCHUNKS = K
PRODF = K * NF          # 50176

USE_DR = False  # DoubleRow is fp8-only on TRN2


def _build(reps=1):
    nc = bacc.Bacc(trn_type="TRN2")

    xs = nc.dram_tensor("xs", [B, C, HP, WP], F32, kind="ExternalInput").ap()
    # DoubleRow layout: xsmm_dr[p, i, n] = x_flat[i*128+p, n], bf16 host-precast
    xsmm = nc.dram_tensor("xsmm", [128, 2 * NALLP], BF16, kind="ExternalInput").ap()
    # w1dr[p, i, m] = w_reduce[m, i*128+p]
    w1d = nc.dram_tensor("w1d", [128, 2 * CR], F32, kind="ExternalInput").ap()
    # rows 0:64 = w_span[perm].T ; row 64 = b2eff[perm]
    w2t = nc.dram_tensor("w2t", [CR + 1, G * KK], F32, kind="ExternalInput").ap()
    out = nc.dram_tensor("out", [128, NF], F32, kind="ExternalOutput").ap()
    # ker bounce scratch in involution order (b, g, kw, p)
    kscr = nc.dram_tensor("kscr", [reps, NCHUNKS, B, G, K, NPIX], BF16).ap()

    with TileContext(nc) as tc:
        with (
            tc.tile_pool(name="const", bufs=1) as cpool,
            tc.tile_pool(name="xp", bufs=1) as xpool,
            tc.tile_pool(name="work", bufs=1) as wpool,
            tc.tile_pool(name="stage", bufs=2) as spool,
            tc.tile_pool(name="psum", bufs=1, space="PSUM") as ppool,
        ):
            # ---------------- weights ----------------
            w1 = cpool.tile([128, 2 * CR], BF16, tag="w1", name="w1")
            nc.gpsimd.dma_start(out=w1[:, :], in_=w1d)
            w1v = w1[:, :].rearrange("p (i m) -> p i m", i=2)

            w2all = cpool.tile([CR + 1, G * KK], BF16, tag="w2", name="w2all")
            nc.gpsimd.dma_start(out=w2all[:, :], in_=w2t)

            # ---------------- x slab (involution layout, partitions=(b,g)) ----
            x_even = xpool.tile([128, XPAD], BF16, tag="xe", name="x_even")
            xs_g = xs.rearrange("b (g c) h w -> (b g) (c h w)", g=G)
            nc.vector.memset(x_even[:, :], 0.0)
            nc.gpsimd.dma_start(out=x_even[:, XOFF:XOFF + XFLAT], in_=xs_g)

            # ---------------- persistent tiles ----------------
            z_sb = wpool.tile([CR + 1, NALLP], BF16, tag="z", name="z_sb")
            nc.vector.memset(z_sb[CR:CR + 1, :], 1.0)
            acc = wpool.tile([128, NF], F32, tag="acc", name="acc")
            tmp = wpool.tile([128, NF], BF16, tag="tmp", name="tmp")
            prod = wpool.tile([128, PRODF], BF16, tag="prod", name="prod")

            # stage-1 rhs aliases the prod buffer (reloaded each rep)
            xmm_flat = prod[:, 0:2 * NALLP]
            xmm_v = prod[:, 0:2 * NALLP].rearrange("p (i n) -> p i n", i=2)

            # two psum halves (4+3 banks): matmuls into one half overlap the
            # activation copy of the other
            psA = ppool.tile([MCHUNK, 2048], F32, tag="psa", name="psA")
            psB = ppool.tile([MCHUNK, 1536], F32, tag="psb", name="psB")
            HALVES = (
                (psA, 0, 2048,
                 ((0, 512), (512, 1024), (1024, 1536), (1536, 2048))),
                (psB, 2048, 1536, ((0, 512), (512, 1024), (1024, 1536))),
            )

            # custom overlapping AP helper for the fused involution mul
            x0 = x_even[:, 0:1]
            part_pair = x_even[:, :].ap[0]  # [stride, 128]

            def xin_ap(kh):
                base = XOFF - PAD + kh * WP
                return bass_rust.AP(
                    tensor=x0.tensor,
                    offset=x0.offset + base,
                    ap=[list(part_pair), [1, K], [CROW, CPG], [1, NPIX]],
                )

            for rep in range(reps):
                # ---------------- load stage-1 rhs ----------------
                nc.sync.dma_start(out=xmm_flat, in_=xsmm)

                # ---------------- z = w1.T @ x ----------------
                for po, base, _w, splits in HALVES:
                    for (a, b_) in splits:
                        for i in range(2):
                            nc.tensor.matmul(
                                out=po[0:CR, a:b_],
                                lhsT=w1v[:, i, :],
                                rhs=xmm_v[:, i, base + a:base + b_],
                                start=(i == 0),
                                stop=(i == 1),
                            )
                nc.scalar.copy(z_sb[0:CR, 0:2048], psA[0:CR, :])
                nc.scalar.copy(z_sb[0:CR, 2048:NALLP], psB[0:CR, :])

                # ---------------- ker chunks + gather ----------------
                ktaps = []
                for j in range(NCHUNKS):
                    lhsT2 = w2all[:, j * MCHUNK:(j + 1) * MCHUNK]
                    kst = spool.tile(
                        [MCHUNK, NALLP], BF16, tag="kst", bufs=2,
                        name=f"kst{rep}_{j}",
                    )
                    for po, base, w, splits in HALVES:
                        for (a, b_) in splits:
                            nc.tensor.matmul(
                                out=po[:, a:b_],
                                lhsT=lhsT2,
                                rhs=z_sb[:, base + a:base + b_],
                                start=True,
                                stop=True,
                            )
                        nc.scalar.copy(kst[:, base:base + w], po[:, :])

                    # one spill to DRAM in involution order (b,g,kw,p),
                    # one fully-contiguous gather back to partitions (b,g)
                    eng_s = nc.sync if j % 2 == 0 else nc.scalar
                    eng_g = nc.scalar if j % 2 == 0 else nc.sync
                    eng_s.dma_start(
                        out=kscr[rep, j].rearrange("b g kw p -> (g kw) b p"),
                        in_=kst[:, :].rearrange("a (b p) -> a b p", b=B),
                    )
                    ktap = spool.tile(
                        [128, K * NPIX], BF16, tag="ktap", bufs=2,
                        name=f"ktap{rep}_{j}",
                    )
                    eng_g.dma_start(
                        out=ktap[:, :],
                        in_=kscr[rep, j].rearrange("b g kw p -> (b g) (kw p)"),
                    )
                    ktaps.append(ktap)

                # ---------------- involution ----------------
                with nc.allow_low_precision("involution bf16 partials"):
                    for kh in range(K):
                        ktap = ktaps[kh]
                        kv = (
                            ktap[:, :]
                            .rearrange("p (kw r) -> p kw r", kw=K)
                            .unsqueeze(2)
                            .broadcast_to((128, K, CPG, NPIX))
                        )
                        pv = prod[:, :].rearrange(
                            "p (kw c r) -> p kw c r", kw=K, c=CPG
                        )
                        nc.vector.tensor_mul(pv, xin_ap(kh), kv)
                        rin = prod[:, :].rearrange("p (kw f) -> p f kw", kw=K)
                        if kh == 0:
                            nc.vector.tensor_reduce(
                                out=acc[:, :], in_=rin,
                                axis=mybir.AxisListType.X,
                                op=mybir.AluOpType.add,
                            )
                        else:
                            nc.vector.tensor_reduce(
                                out=tmp[:, :], in_=rin,
                                axis=mybir.AxisListType.X,
                                op=mybir.AluOpType.add,
                            )
                            nc.vector.tensor_add(acc[:, :], acc[:, :], tmp[:, :])

                # ---------------- store ----------------
                nc.scalar.dma_start(out=out, in_=acc[:, :])

    return nc


_CACHE = {}


def _get_program(reps=1):
    if reps not in _CACHE:
        nc = _build(reps)
        nc.compile()
        _CACHE[reps] = nc
    return _CACHE[reps]


def _make_inputs(x, w_reduce, b_reduce, w_span, b_span):
    import ml_dtypes

    x = np.ascontiguousarray(np.asarray(x, dtype=np.float32))
    w_reduce = np.asarray(w_reduce, np.float32)
    b_reduce = np.asarray(b_reduce, np.float32)
    w_span = np.asarray(w_span, np.float32)
    b_span = np.asarray(b_span, np.float32)

    # w1 DoubleRow layout: w1d[p, i*CR + m] = w_reduce[m, i*128+p]
    w1d = np.ascontiguousarray(
        w_reduce.T.reshape(2, 128, CR).transpose(1, 0, 2).reshape(128, 2 * CR)
    )

    # tap-major permutation: col j*112 + g*7 + kk <- w_span row g*49 + j*7 + kk
    perm = np.empty(G * KK, np.int64)
    idx = 0
    for j in range(NCHUNKS):
        for g in range(G):
            for kk in range(K):
                perm[idx] = g * KK + j * K + kk
                idx += 1
    b2eff = w_span @ b_reduce + b_span
    w2te = np.concatenate([w_span[perm].T, b2eff[perm][None, :]], axis=0)
    w2te = np.ascontiguousarray(w2te)

    in_maps = []
    for i in range(NCORES):
        h0 = i * HS - HALO
        sl = np.zeros((B, C, HP, WP), np.float32)
        s0, s1 = max(0, h0), min(H, h0 + HP)
        sl[:, :, s0 - h0:s1 - h0, LPAD:LPAD + W] = x[:, :, s0:s1, :]
        xf = sl[:, :, HALO:HALO + HS, :].transpose(1, 0, 2, 3).reshape(C, NALLP)
        # DoubleRow rhs layout: [p, i*NALLP + n] = xf[i*128+p, n], bf16
        xdr = np.ascontiguousarray(
            xf.reshape(2, 128, NALLP).transpose(1, 0, 2).reshape(128, 2 * NALLP)
        ).astype(ml_dtypes.bfloat16)
        in_maps.append({"xs": sl, "xsmm": xdr, "w1d": w1d, "w2t": w2te})
    return in_maps


def _unpack_out(arr):
    """[128=(b,g), NF=(c',hs,wp)] fp32 -> [B, C, HS, W]"""
    a = arr.reshape(B, C, HS, WP)[:, :, :, LPAD:LPAD + W]
    return np.ascontiguousarray(a)


def kernel_with_results(x, w_reduce, b_reduce, w_span, b_span, trace=False, reps=1):
    in_maps = _make_inputs(x, w_reduce, b_reduce, w_span, b_span)
    nc = _get_program(reps)
    res = run_bass_kernel_spmd(nc, in_maps, list(range(NCORES)), trace=trace)
    full = np.concatenate(
        [_unpack_out(res.results[i]["out"]) for i in range(NCORES)], axis=2
    ).astype(np.float32)
    return full, res


def kernel(x, w_reduce, b_reduce, w_span, b_span):
    full, _ = kernel_with_results(x, w_reduce, b_reduce, w_span, b_span)
    return full


# revision 5
# speedup vs baseline: 5.1530x; 3.3880x over previous
"""Involution2d (B=8, C=256, H=W=56, K=7, G=16, reduction=4) on 8 TRN2 cores, v2.

Same layout as v1 (spatial shard over H, partitions=(g,b), padded width 64) but
restructured to minimize instruction count and dependency depth:
  - stage-1 bias folded into stage-2 via b2eff = w_span@b_reduce + b_span;
    stage-2 bias folded into the matmul via a ones-row in z (contraction 65).
  - stage-1 uses DoubleRow perf mode: contraction 256 in one matmul -> 7 mm.
  - stage-2: 7 chunks x 7 matmuls into 2048+1536 psum tiles, 2 copies each.
  - ktap gather is a single SBUF->SBUF DMA per (chunk, half) - no DRAM trip.
  - involution per kh: ONE fused mul [128,7,16,448] with an overlapping
    x AP + ONE strided tensor_reduce over kw + acc add  (3 vector ops/kh
    instead of 15).
  - xmm (stage-1 rhs, bf16 host-precast) aliases the prod buffer and is
    reloaded per rep (1 contiguous DMA).
"""

import os
import sys

import numpy as np

for _p in ("/opt/trn_rl_repo",):
    if os.path.isdir(_p) and _p not in sys.path:
        sys.path.insert(0, _p)

import bass_rust
import concourse.bacc as bacc
import concourse.mybir as mybir
from concourse.tile import TileContext
from concourse.bass_utils import run_bass_kernel_spmd

B, C, H, W = 8, 256, 56, 56
G, K, PAD = 16, 7, 3
CPG = C // G            # 16
KK = K * K
CR = 64
NCORES = 8
HS = H // NCORES        # 7 rows per core
HALO = PAD
HP = HS + 2 * HALO      # 13
LPAD = 4
WP = 64                 # 4 + 56 + 4
NPIX = HS * WP          # 448
NALLP = B * NPIX        # 3584
CROW = HP * WP          # 832
XFLAT = CPG * CROW      # 13312
XOFF = 8
XPAD = 13712            # XOFF + XFLAT + max shift (387) + slack
NF = CPG * NPIX         # 7168

F32 = mybir.dt.float32
BF16 = mybir.dt.bfloat16

MCHUNK = G * K          # 112
NCHUNKS = K
PRODF = K * NF          # 50176

USE_DR = False  # DoubleRow is fp8-only on TRN2


def _build(reps=1):
    nc = bacc.Bacc(trn_type="TRN2")

    xs = nc.dram_tensor("xs", [B, C, HP, WP], F32, kind="ExternalInput").ap()
    # DoubleRow layout: xsmm_dr[p, i, n] = x_flat[i*128+p, n], bf16 host-precast
    xsmm = nc.dram_tensor("xsmm", [128, 2 * NALLP], BF16, kind="ExternalInput").ap()
    # w1dr[p, i, m] = w_reduce[m, i*128+p]
    w1d = nc.dram_tensor("w1d", [128, 2 * CR], F32, kind="ExternalInput").ap()
    # rows 0:64 = w_span[perm].T ; row 64 = b2eff[perm]
    w2t = nc.dram_tensor("w2t", [CR + 1, G * KK], F32, kind="ExternalInput").ap()
    out = nc.dram_tensor("out", [128, NF], F32, kind="ExternalOutput").ap()
    # ker bounce scratch in involution order (b, g, kw, p)
    kscr = nc.dram_tensor("kscr", [reps, NCHUNKS, B, G, K, NPIX], BF16).ap()

    with TileContext(nc) as tc:
        with (
            tc.tile_pool(name="const", bufs=1) as cpool,
            tc.tile_pool(name="xp", bufs=1) as xpool,
            tc.tile_pool(name="work", bufs=1) as wpool,
            tc.tile_pool(name="stage", bufs=2) as spool,
            tc.tile_pool(name="psum", bufs=1, space="PSUM") as ppool,
        ):
            # ---------------- weights ----------------
            w1 = cpool.tile([128, 2 * CR], BF16, tag="w1", name="w1")
            nc.gpsimd.dma_start(out=w1[:, :], in_=w1d)
            w1v = w1[:, :].rearrange("p (i m) -> p i m", i=2)

            w2all = cpool.tile([CR + 1, G * KK], BF16, tag="w2", name="w2all")
            nc.gpsimd.dma_start(out=w2all[:, :], in_=w2t)

            # ---------------- x slab (involution layout, partitions=(b,g)) ----
            x_even = xpool.tile([128, XPAD], BF16, tag="xe", name="x_even")
            xs_g = xs.rearrange("b (g c) h w -> (b g) (c h w)", g=G)
            nc.vector.memset(x_even[:, :], 0.0)
            nc.gpsimd.dma_start(out=x_even[:, XOFF:XOFF + XFLAT], in_=xs_g)

            # ---------------- persistent tiles ----------------
            z_sb = wpool.tile([CR + 1, NALLP], BF16, tag="z", name="z_sb")
            nc.vector.memset(z_sb[CR:CR + 1, :], 1.0)
            # 9 planes: 0/8 = bf16 acc ping-pong (plane 0 doubles as the
            # stage-1 rhs buffer), 1-7 = per-kw products
            prod = wpool.tile([128, 9 * NF], BF16, tag="prod", name="prod")

            # stage-1 rhs aliases the prod buffer (reloaded each rep)
            xmm_flat = prod[:, 0:2 * NALLP]
            xmm_v = prod[:, 0:2 * NALLP].rearrange("p (i n) -> p i n", i=2)

            # two psum halves (4+3 banks): matmuls into one half overlap the
            # activation copy of the other
            psA = ppool.tile([MCHUNK, 2048], F32, tag="psa", name="psA")
            psB = ppool.tile([MCHUNK, 1536], F32, tag="psb", name="psB")
            HALVES = (
                (psA, 0, 2048,
                 ((0, 512), (512, 1024), (1024, 1536), (1536, 2048))),
                (psB, 2048, 1536, ((0, 512), (512, 1024), (1024, 1536))),
            )

            # custom overlapping AP helper for the fused involution mul
            x0 = x_even[:, 0:1]
            part_pair = x_even[:, :].ap[0]  # [stride, 128]

            def xin_ap(kh):
                base = XOFF - PAD + kh * WP
                return bass_rust.AP(
                    tensor=x0.tensor,
                    offset=x0.offset + base,
                    ap=[list(part_pair), [1, K], [CROW, CPG], [1, NPIX]],
                )

            for rep in range(reps):
                # ---------------- load stage-1 rhs ----------------
                nc.sync.dma_start(out=xmm_flat, in_=xsmm)

                # ---------------- z = w1.T @ x ----------------
                for po, base, _w, splits in HALVES:
                    for (a, b_) in splits:
                        for i in range(2):
                            nc.tensor.matmul(
                                out=po[0:CR, a:b_],
                                lhsT=w1v[:, i, :],
                                rhs=xmm_v[:, i, base + a:base + b_],
                                start=(i == 0),
                                stop=(i == 1),
                            )
                nc.scalar.copy(z_sb[0:CR, 0:2048], psA[0:CR, :])
                nc.scalar.copy(z_sb[0:CR, 2048:NALLP], psB[0:CR, :])

                # ---------------- ker chunks + gather ----------------
                ktaps = []
                for j in range(NCHUNKS):
                    lhsT2 = w2all[:, j * MCHUNK:(j + 1) * MCHUNK]
                    kst = spool.tile(
                        [MCHUNK, NALLP], BF16, tag="kst", bufs=2,
                        name=f"kst{rep}_{j}",
                    )
                    for po, base, w, splits in HALVES:
                        for (a, b_) in splits:
                            nc.tensor.matmul(
                                out=po[:, a:b_],
                                lhsT=lhsT2,
                                rhs=z_sb[:, base + a:base + b_],
                                start=True,
                                stop=True,
                            )
                        nc.scalar.copy(kst[:, base:base + w], po[:, :])

                    # one spill to DRAM in involution order (b,g,kw,p),
                    # one fully-contiguous gather back to partitions (b,g)
                    eng_s = nc.sync if j % 2 == 0 else nc.scalar
                    eng_g = nc.scalar if j % 2 == 0 else nc.sync
                    eng_s.dma_start(
                        out=kscr[rep, j].rearrange("b g kw p -> (g kw) b p"),
                        in_=kst[:, :].rearrange("a (b p) -> a b p", b=B),
                    )
                    ktap = spool.tile(
                        [128, K * NPIX], BF16, tag="ktap", bufs=2,
                        name=f"ktap{rep}_{j}",
                    )
                    eng_g.dma_start(
                        out=ktap[:, :],
                        in_=kscr[rep, j].rearrange("b g kw p -> (b g) (kw p)"),
                    )
                    ktaps.append(ktap)

                # ---------------- involution ----------------
                with nc.allow_low_precision("involution bf16 partials"):
                    for kh in range(K):
                        ktap = ktaps[kh]
                        kv = (
                            ktap[:, :]
                            .rearrange("p (kw r) -> p kw r", kw=K)
                            .unsqueeze(2)
                            .broadcast_to((128, K, CPG, NPIX))
                        )
                        pv = prod[:, NF:8 * NF].rearrange(
                            "p (kw c r) -> p kw c r", kw=K, c=CPG
                        )
                        nc.vector.tensor_mul(pv, xin_ap(kh), kv)
                        if kh == 0:
                            rin = prod[:, NF:8 * NF].rearrange(
                                "p (kw f) -> p f kw", kw=K
                            )
                            ro = prod[:, 8 * NF:9 * NF]
                        elif kh % 2 == 1:
                            rin = prod[:, NF:9 * NF].rearrange(
                                "p (kw f) -> p f kw", kw=8
                            )
                            ro = prod[:, 0:NF]
                        else:
                            rin = prod[:, 0:8 * NF].rearrange(
                                "p (kw f) -> p f kw", kw=8
                            )
                            ro = prod[:, 8 * NF:9 * NF]
                        nc.vector.tensor_reduce(
                            out=ro, in_=rin,
                            axis=mybir.AxisListType.X,
                            op=mybir.AluOpType.add,
                        )

                # ---------------- store (bf16 acc plane 8, casting DMA) ----
                nc.gpsimd.dma_start(out=out, in_=prod[:, 8 * NF:9 * NF])

    return nc


_CACHE = {}


def _get_program(reps=1):
    if reps not in _CACHE:
        nc = _build(reps)
        nc.compile()
        _CACHE[reps] = nc
    return _CACHE[reps]


def _make_inputs(x, w_reduce, b_reduce, w_span, b_span):
    import ml_dtypes

    x = np.ascontiguousarray(np.asarray(x, dtype=np.float32))
    w_reduce = np.asarray(w_reduce, np.float32)
    b_reduce = np.asarray(b_reduce, np.float32)
    w_span = np.asarray(w_span, np.float32)
    b_span = np.asarray(b_span, np.float32)

    # w1 DoubleRow layout: w1d[p, i*CR + m] = w_reduce[m, i*128+p]
    w1d = np.ascontiguousarray(
        w_reduce.T.reshape(2, 128, CR).transpose(1, 0, 2).reshape(128, 2 * CR)
    )

    # tap-major permutation: col j*112 + g*7 + kk <- w_span row g*49 + j*7 + kk
    perm = np.empty(G * KK, np.int64)
    idx = 0
    for j in range(NCHUNKS):
        for g in range(G):
            for kk in range(K):
                perm[idx] = g * KK + j * K + kk
                idx += 1
    b2eff = w_span @ b_reduce + b_span
    w2te = np.concatenate([w_span[perm].T, b2eff[perm][None, :]], axis=0)
    w2te = np.ascontiguousarray(w2te)

    in_maps = []
    for i in range(NCORES):
        h0 = i * HS - HALO
        sl = np.zeros((B, C, HP, WP), np.float32)
        s0, s1 = max(0, h0), min(H, h0 + HP)
        sl[:, :, s0 - h0:s1 - h0, LPAD:LPAD + W] = x[:, :, s0:s1, :]
        xf = sl[:, :, HALO:HALO + HS, :].transpose(1, 0, 2, 3).reshape(C, NALLP)
        # DoubleRow rhs layout: [p, i*NALLP + n] = xf[i*128+p, n], bf16
        xdr = np.ascontiguousarray(
            xf.reshape(2, 128, NALLP).transpose(1, 0, 2).reshape(128, 2 * NALLP)
        ).astype(ml_dtypes.bfloat16)
        in_maps.append({"xs": sl, "xsmm": xdr, "w1d": w1d, "w2t": w2te})
    return in_maps


def _unpack_out(arr):
    """[128=(b,g), NF=(c',hs,wp)] fp32 -> [B, C, HS, W]"""
    a = arr.reshape(B, C, HS, WP)[:, :, :, LPAD:LPAD + W]
    return np.ascontiguousarray(a)


def kernel_with_results(x, w_reduce, b_reduce, w_span, b_span, trace=False, reps=1):
    in_maps = _make_inputs(x, w_reduce, b_reduce, w_span, b_span)
    nc = _get_program(reps)
    res = run_bass_kernel_spmd(nc, in_maps, list(range(NCORES)), trace=trace)
    full = np.concatenate(
        [_unpack_out(res.results[i]["out"]) for i in range(NCORES)], axis=2
    ).astype(np.float32)
    return full, res


def kernel(x, w_reduce, b_reduce, w_span, b_span):
    full, _ = kernel_with_results(x, w_reduce, b_reduce, w_span, b_span)
    return full
